# revision 8
# baseline (speedup 1.0000x reference)
"""GAT segment-softmax reduce (nn_GATReduce) for 8 Trainium2 NeuronCores.

Strategy:
  - Host: sort edges by dst (CSR-ization), fold the a1[dst] gather into a
    per-edge score s = a1[dst] + a2, split nodes into 8 contiguous ranges
    (49 blocks of 128 nodes each); every core fully owns its node range so
    no collectives are needed.
  - Softmax without segment-max: inputs are bounded (|s| < ~10) so
    exp(lrelu(s)) is safe in fp32 and softmax is shift-invariant.
  - Device (per core): for each 128-node block, K edge tiles of 128 sorted
    edges. Per tile: one-hot [128e,128n] via one DVE tensor_scalar(is_equal)
    against an iota constant; vals = ex * ft via per-head tensor_scalar_mul;
    segment reduction = PE matmul onehot.T @ vals accumulated in PSUM.
    Pad edges get s = -1e9 -> ex = exp(-1e7) = 0, contributing nothing.
"""

import math
import os

import numpy as np

import concourse.bacc as bacc
import concourse.bass as bass
import concourse.mybir as mybir
import concourse.tile as tile
from concourse.bass_utils import run_bass_kernel_spmd

P = 128          # partition count / node block size / edge tile size
H = 4            # heads
D = 64           # feature dim
HD = H * D       # 256
N_CORES = 8

_kernel_cache = {}
LAST_RESULT = None
LAST_NC = None
LAST_IN_MAPS = None


def _build(nblk: int, k: int):
    """Build the single-core Bass program (SPMD across 8 cores)."""
    nc = bacc.Bacc("TRN2", target_bir_lowering=False, debug=False)
    f32 = mybir.dt.float32

    ft_i = nc.dram_tensor("ft_i", [nblk * k * P, HD], f32, kind="ExternalInput")
    s_i = nc.dram_tensor("s_i", [nblk, P, k * H], f32, kind="ExternalInput")
    d_i = nc.dram_tensor("d_i", [nblk, P, k], f32, kind="ExternalInput")
    iota_i = nc.dram_tensor("iota_i", [P, P], f32, kind="ExternalInput")
    out_o = nc.dram_tensor("out_o", [nblk * P, HD], f32, kind="ExternalOutput")

    # DRAM view of ft as [nblk, P(partition), k, HD]
    ft_v = ft_i[:].rearrange("(b t p) d -> b p t d", t=k, p=P)

    with tile.TileContext(nc) as tc:
        with (
            tc.tile_pool(name="const", bufs=1) as cp,
            tc.tile_pool(name="ftp", bufs=3) as ftp,
            tc.tile_pool(name="meta", bufs=3) as mp,
            tc.tile_pool(name="work", bufs=3) as wp,
            tc.tile_pool(name="ohp", bufs=4) as ohp,
            tc.tile_pool(name="valp", bufs=4) as vp,
            tc.tile_pool(name="outp", bufs=3) as op_,
            tc.tile_pool(name="psum", bufs=2, space="PSUM") as pp,
        ):
            iota_t = cp.tile([P, P], f32)
            nc.sync.dma_start(out=iota_t[:], in_=iota_i[:])

            for b in range(nblk):
                ft_blk = ftp.tile([P, k, HD], f32)
                nc.sync.dma_start(out=ft_blk[:], in_=ft_v[b])
                s_blk = mp.tile([P, k * H], f32)
                nc.sync.dma_start(out=s_blk[:], in_=s_i[b])
                d_blk = mp.tile([P, k], f32)
                nc.sync.dma_start(out=d_blk[:], in_=d_i[b])

                # ex = exp(leaky_relu(s)) for the whole block's edges
                m_t = wp.tile([P, k * H], f32)
                nc.vector.tensor_scalar_mul(m_t[:], s_blk[:], 0.01)
                e_t = wp.tile([P, k * H], f32)
                nc.vector.tensor_tensor(
                    out=e_t[:], in0=s_blk[:], in1=m_t[:], op=mybir.AluOpType.max
                )
                ex_blk = wp.tile([P, k * H], f32)
                nc.scalar.activation(
                    ex_blk[:], e_t[:], mybir.ActivationFunctionType.Exp
                )

                acc_num = pp.tile([P, HD], f32, tag="acc_num")
                acc_den = pp.tile([P, H], f32, tag="acc_den")
                for t in range(k):
                    oh = ohp.tile([P, P], f32)
                    nc.vector.tensor_scalar(
                        out=oh[:],
                        in0=iota_t[:],
                        scalar1=d_blk[:, t : t + 1],
                        scalar2=None,
                        op0=mybir.AluOpType.is_equal,
                    )
                    vals = vp.tile([P, HD], f32)
                    for h in range(H):
                        nc.vector.tensor_scalar_mul(
                            vals[:, h * D : (h + 1) * D],
                            ft_blk[:, t, h * D : (h + 1) * D],
                            ex_blk[:, t * H + h : t * H + h + 1],
                        )
                    nc.tensor.matmul(
                        acc_num[:], lhsT=oh[:], rhs=vals[:],
                        start=(t == 0), stop=(t == k - 1),
                    )
                    nc.tensor.matmul(
                        acc_den[:], lhsT=oh[:],
                        rhs=ex_blk[:, t * H : (t + 1) * H],
                        start=(t == 0), stop=(t == k - 1),
                    )

                den = wp.tile([P, H], f32, tag="den")
                nc.vector.tensor_scalar_add(den[:], acc_den[:], 1e-30)
                rec = wp.tile([P, H], f32, tag="rec")
                nc.vector.reciprocal(rec[:], den[:])
                outsb = op_.tile([P, HD], f32)
                for h in range(H):
                    nc.vector.tensor_scalar_mul(
                        outsb[:, h * D : (h + 1) * D],
                        acc_num[:, h * D : (h + 1) * D],
                        rec[:, h : h + 1],
                    )
                nc.sync.dma_start(
                    out=out_o[b * P : (b + 1) * P, :], in_=outsb[:]
                )

    nc.compile()
    return nc


def kernel(a1, a2, ft, dst):
    global LAST_RESULT, LAST_NC, LAST_IN_MAPS
    a1 = np.asarray(a1, dtype=np.float32)
    a2 = np.asarray(a2, dtype=np.float32)
    ft = np.asarray(ft, dtype=np.float32)
    dst = np.asarray(dst)

    n = a1.shape[0]
    e = dst.shape[0]
    assert a1.shape == (n, H, 1) and a2.shape == (e, H, 1)
    assert ft.shape == (e, H, D)

    # ---- host prep: sort edges by dst, fold a1 gather ----
    order = np.argsort(dst, kind="stable")
    dst_s = dst[order].astype(np.int64)
    s_all = (a1[:, :, 0][dst_s] + a2[order, :, 0]).astype(np.float32)  # [E,H]
    ft_s = ft[order].reshape(e, HD)  # [E, 256]

    nblk_total = math.ceil(n / P)                      # 391
    nblk = math.ceil(nblk_total / N_CORES)             # 49 blocks per core
    npc = nblk * P                                     # 6272 nodes per core

    # edges per 128-node block (global)
    block_starts = np.searchsorted(dst_s, np.arange(0, (nblk * N_CORES) * P + 1, P))
    counts = np.diff(block_starts)                     # [nblk*8]
    k = max(1, int(math.ceil(counts.max() / P)))       # edge tiles per block
    epb = k * P                                        # padded edges per block

    # ---- pack per-core inputs ----
    iota_np = np.broadcast_to(
        np.arange(P, dtype=np.float32)[None, :], (P, P)
    ).copy()

    in_maps = []
    for c in range(N_CORES):
        ftp = np.zeros((nblk * epb, HD), dtype=np.float32)
        sp = np.full((nblk * epb, H), -1e9, dtype=np.float32)
        dp = np.zeros((nblk * epb,), dtype=np.float32)
        for bl in range(nblk):
            g = c * nblk + bl                          # global block id
            lo, hi = block_starts[g], block_starts[g + 1]
            cnt = hi - lo
            o = bl * epb
            ftp[o : o + cnt] = ft_s[lo:hi]
            sp[o : o + cnt] = s_all[lo:hi]
            dp[o : o + cnt] = (dst_s[lo:hi] - g * P).astype(np.float32)
        # swizzle: [nblk, k, P, x] -> [nblk, P, k, x]
        s_sw = np.ascontiguousarray(
            sp.reshape(nblk, k, P, H).transpose(0, 2, 1, 3)
        ).reshape(nblk, P, k * H)
        d_sw = np.ascontiguousarray(
            dp.reshape(nblk, k, P).transpose(0, 2, 1)
        )
        in_maps.append({"ft_i": ftp, "s_i": s_sw, "d_i": d_sw, "iota_i": iota_np})

    key = (nblk, k)
    if key not in _kernel_cache:
        _kernel_cache[key] = _build(nblk, k)
    nc = _kernel_cache[key]

    res = run_bass_kernel_spmd(nc, in_maps, core_ids=list(range(N_CORES)))
    LAST_RESULT = res
    LAST_NC = nc
    LAST_IN_MAPS = in_maps

    out = np.empty((n, H, D), dtype=np.float32)
    for c in range(N_CORES):
        lo = c * npc
        real = min(npc, n - lo)
        if real <= 0:
            break
        out[lo : lo + real] = res.results[c]["out_o"].reshape(npc, H, D)[:real]
    return out


# revision 11
# speedup vs baseline: 1.2179x; 1.2179x over previous
"""GAT segment-softmax reduce (nn_GATReduce) for 8 Trainium2 NeuronCores.

Strategy:
  - Host: sort edges by dst (CSR-ization), fold the a1[dst] gather into a
    per-edge score s = a1[dst] + a2, split nodes into 8 contiguous ranges
    (49 blocks of 128 nodes each); every core fully owns its node range so
    no collectives are needed.
  - Softmax without segment-max: inputs are bounded (|s| < ~10) so
    exp(lrelu(s)) is safe in fp32 and softmax is shift-invariant.
  - Device (per core): for each 128-node block, K edge tiles of 128 sorted
    edges. Per tile: one-hot [128e,128n] via one DVE tensor_scalar(is_equal)
    against an iota constant; vals = ex * ft via per-head tensor_scalar_mul;
    segment reduction = PE matmul onehot.T @ vals accumulated in PSUM.
    Pad edges get s = -1e9 -> ex = exp(-1e7) = 0, contributing nothing.
"""

import math
import os

import numpy as np

import concourse.bacc as bacc
import concourse.bass as bass
import concourse.mybir as mybir
import concourse.tile as tile
from concourse.bass_utils import run_bass_kernel_spmd

P = 128          # partition count / node block size / edge tile size
H = 4            # heads
D = 64           # feature dim
HD = H * D       # 256
N_CORES = 8

_kernel_cache = {}
LAST_RESULT = None
LAST_NC = None
LAST_IN_MAPS = None


def _build(nblk: int, k: int, reps: int = 1):
    """Build the single-core Bass program (SPMD across 8 cores).

    DVE work is batched per node block (not per edge tile) using
    broadcast access patterns, since per-op overhead dominates DVE cost.
    `reps` repeats the whole workload inside one NEFF (for timing).
    """
    nc = bacc.Bacc("TRN2", target_bir_lowering=False, debug=False)
    f32 = mybir.dt.float32

    ft_i = nc.dram_tensor("ft_i", [nblk * k * P, HD], f32, kind="ExternalInput")
    s_i = nc.dram_tensor("s_i", [nblk, P, k * H], f32, kind="ExternalInput")
    d_i = nc.dram_tensor("d_i", [nblk, P, k], f32, kind="ExternalInput")
    iota_i = nc.dram_tensor("iota_i", [P, P], f32, kind="ExternalInput")
    out_o = nc.dram_tensor("out_o", [nblk * P, HD], f32, kind="ExternalOutput")

    # DRAM view of ft as [nblk, P(partition), k, HD]
    ft_v = ft_i[:].rearrange("(b t p) d -> b p t d", t=k, p=P)

    with tile.TileContext(nc) as tc:
        with (
            tc.tile_pool(name="const", bufs=1) as cp,
            tc.tile_pool(name="ftp", bufs=3) as ftp,
            tc.tile_pool(name="meta", bufs=3) as mp,
            tc.tile_pool(name="work", bufs=3) as wp,
            tc.tile_pool(name="ohp", bufs=3) as ohp,
            tc.tile_pool(name="valp", bufs=3) as vp,
            tc.tile_pool(name="outp", bufs=3) as op_,
            tc.tile_pool(name="psum", bufs=4, space="PSUM") as pp,
        ):
            iota_t = cp.tile([P, P], f32)
            nc.sync.dma_start(out=iota_t[:], in_=iota_i[:])

            for _rep in range(reps):
                for b in range(nblk):
                    ft_blk = ftp.tile([P, k, HD], f32)
                    nc.sync.dma_start(out=ft_blk[:], in_=ft_v[b])
                    s_blk = mp.tile([P, k * H], f32)
                    nc.sync.dma_start(out=s_blk[:], in_=s_i[b])
                    d_blk = mp.tile([P, k], f32)
                    nc.sync.dma_start(out=d_blk[:], in_=d_i[b])

                    # ex = exp(leaky_relu(s)) for the whole block's edges
                    e_t = wp.tile([P, k * H], f32)
                    nc.vector.scalar_tensor_tensor(
                        out=e_t[:], in0=s_blk[:], scalar=0.01, in1=s_blk[:],
                        op0=mybir.AluOpType.mult, op1=mybir.AluOpType.max,
                    )
                    ex_blk = wp.tile([P, k * H], f32)
                    nc.scalar.activation(
                        ex_blk[:], e_t[:], mybir.ActivationFunctionType.Exp
                    )

                    # one-hot for all k tiles in one op:
                    # oh[e, t, n] = (iota[n] == dstl[e, t])
                    oh_blk = ohp.tile([P, k, P], f32)
                    nc.vector.tensor_tensor(
                        out=oh_blk[:],
                        in0=iota_t[:, None, :].to_broadcast([P, k, P]),
                        in1=d_blk[:, :, None].to_broadcast([P, k, P]),
                        op=mybir.AluOpType.is_equal,
                    )

                    # vals[e, t, h, d] = ft[e, t, h, d] * ex[e, t, h]
                    vals_blk = vp.tile([P, k, H, D], f32)
                    nc.vector.tensor_tensor(
                        out=vals_blk[:],
                        in0=ft_blk[:].rearrange("p t (h d) -> p t h d", h=H),
                        in1=ex_blk[:].rearrange("p (t h) -> p t h", h=H)[
                            :, :, :, None
                        ].to_broadcast([P, k, H, D]),
                        op=mybir.AluOpType.mult,
                    )

                    acc_num = pp.tile([P, HD], f32, tag="acc_num")
                    acc_den = pp.tile([P, H], f32, tag="acc_den")
                    for t in range(k):
                        nc.tensor.matmul(
                            acc_num[:], lhsT=oh_blk[:, t, :], rhs=vals_blk[:, t],
                            start=(t == 0), stop=(t == k - 1),
                        )
                        nc.tensor.matmul(
                            acc_den[:], lhsT=oh_blk[:, t, :],
                            rhs=ex_blk[:, t * H : (t + 1) * H],
                            start=(t == 0), stop=(t == k - 1),
                        )

                    den = wp.tile([P, H], f32, tag="den")
                    nc.vector.tensor_scalar_add(den[:], acc_den[:], 1e-30)
                    rec = wp.tile([P, H], f32, tag="rec")
                    nc.vector.reciprocal(rec[:], den[:])
                    outsb = op_.tile([P, H, D], f32)
                    nc.vector.tensor_tensor(
                        out=outsb[:],
                        in0=acc_num[:].rearrange("p (h d) -> p h d", h=H),
                        in1=rec[:, :, None].to_broadcast([P, H, D]),
                        op=mybir.AluOpType.mult,
                    )
                    nc.sync.dma_start(
                        out=out_o[b * P : (b + 1) * P, :],
                        in_=outsb[:].rearrange("p h d -> p (h d)"),
                    )

    nc.compile()
    return nc


def kernel(a1, a2, ft, dst):
    global LAST_RESULT, LAST_NC, LAST_IN_MAPS
    a1 = np.asarray(a1, dtype=np.float32)
    a2 = np.asarray(a2, dtype=np.float32)
    ft = np.asarray(ft, dtype=np.float32)
    dst = np.asarray(dst)

    n = a1.shape[0]
    e = dst.shape[0]
    assert a1.shape == (n, H, 1) and a2.shape == (e, H, 1)
    assert ft.shape == (e, H, D)

    # ---- host prep: sort edges by dst, fold a1 gather ----
    order = np.argsort(dst, kind="stable")
    dst_s = dst[order].astype(np.int64)
    s_all = (a1[:, :, 0][dst_s] + a2[order, :, 0]).astype(np.float32)  # [E,H]
    ft_s = ft[order].reshape(e, HD)  # [E, 256]

    nblk_total = math.ceil(n / P)                      # 391
    nblk = math.ceil(nblk_total / N_CORES)             # 49 blocks per core
    npc = nblk * P                                     # 6272 nodes per core

    # edges per 128-node block (global)
    block_starts = np.searchsorted(dst_s, np.arange(0, (nblk * N_CORES) * P + 1, P))
    counts = np.diff(block_starts)                     # [nblk*8]
    k = max(1, int(math.ceil(counts.max() / P)))       # edge tiles per block
    epb = k * P                                        # padded edges per block

    # ---- pack per-core inputs ----
    iota_np = np.broadcast_to(
        np.arange(P, dtype=np.float32)[None, :], (P, P)
    ).copy()

    in_maps = []
    for c in range(N_CORES):
        ftp = np.zeros((nblk * epb, HD), dtype=np.float32)
        sp = np.full((nblk * epb, H), -1e9, dtype=np.float32)
        dp = np.zeros((nblk * epb,), dtype=np.float32)
        for bl in range(nblk):
            g = c * nblk + bl                          # global block id
            lo, hi = block_starts[g], block_starts[g + 1]
            cnt = hi - lo
            o = bl * epb
            ftp[o : o + cnt] = ft_s[lo:hi]
            sp[o : o + cnt] = s_all[lo:hi]
            dp[o : o + cnt] = (dst_s[lo:hi] - g * P).astype(np.float32)
        # swizzle: [nblk, k, P, x] -> [nblk, P, k, x]
        s_sw = np.ascontiguousarray(
            sp.reshape(nblk, k, P, H).transpose(0, 2, 1, 3)
        ).reshape(nblk, P, k * H)
        d_sw = np.ascontiguousarray(
            dp.reshape(nblk, k, P).transpose(0, 2, 1)
        )
        in_maps.append({"ft_i": ftp, "s_i": s_sw, "d_i": d_sw, "iota_i": iota_np})

    key = (nblk, k)
    if key not in _kernel_cache:
        _kernel_cache[key] = _build(nblk, k)
    nc = _kernel_cache[key]

    res = run_bass_kernel_spmd(nc, in_maps, core_ids=list(range(N_CORES)))
    LAST_RESULT = res
    LAST_NC = nc
    LAST_IN_MAPS = in_maps

    out = np.empty((n, H, D), dtype=np.float32)
    for c in range(N_CORES):
        lo = c * npc
        real = min(npc, n - lo)
        if real <= 0:
            break
        out[lo : lo + real] = res.results[c]["out_o"].reshape(npc, H, D)[:real]
    return out


# revision 25
# speedup vs baseline: 4.9583x; 4.0712x over previous
"""GAT segment-softmax reduce (nn_GATReduce) for 8 Trainium2 NeuronCores.

Strategy:
  - Host: sort edges by dst (CSR-ization), fold the a1[dst] gather into a
    per-edge score s = a1[dst] + a2, split nodes into 8 contiguous ranges
    (49 blocks of 128 nodes each); every core fully owns its node range so
    no collectives are needed.
  - Softmax without segment-max: inputs are bounded (|s| < ~10) so
    exp(lrelu(s)) is safe in fp32 and softmax is shift-invariant.
  - Device (per core): for each 128-node block, K edge tiles of 128 sorted
    edges. Per tile: one-hot [128e,128n] via one DVE tensor_scalar(is_equal)
    against an iota constant; vals = ex * ft via per-head tensor_scalar_mul;
    segment reduction = PE matmul onehot.T @ vals accumulated in PSUM.
    Pad edges get s = -1e9 -> ex = exp(-1e7) = 0, contributing nothing.
"""

import math
import os

import numpy as np

import concourse.bacc as bacc
import concourse.bass as bass
import concourse.mybir as mybir
import concourse.tile as tile
from concourse.bass_utils import run_bass_kernel_spmd

P = 128          # partition count / node block size / edge tile size
H = 4            # heads
D = 64           # feature dim
HD = H * D       # 256
N_CORES = 8

_kernel_cache = {}
LAST_RESULT = None
LAST_NC = None
LAST_IN_MAPS = None

# kernel variant flags (must match between _build and input packing)
OH_BF16 = False
GP_TILES = 4


def _build(nblk: int, k: int, reps: int = 1, bf16_oh: bool = False,
           gp_tiles: int = 0, act_lrelu: bool = False):
    """Build the single-core Bass program (SPMD across 8 cores).

    DVE work is batched per node block (not per edge tile) using
    broadcast access patterns, since per-op overhead dominates DVE cost.
    `reps` repeats the whole workload inside one NEFF (for timing).
    `bf16_oh`: feed the is_equal compare bf16 inputs (exact for 0..127).
    `gp_tiles`: offload the vals multiply for the last `gp_tiles` edge
    tiles of each block to GPSIMD.
    """
    nc = bacc.Bacc("TRN2", target_bir_lowering=False, debug=False)
    f32 = mybir.dt.float32
    cmp_dt = mybir.dt.bfloat16 if bf16_oh else f32

    ft_i = nc.dram_tensor("ft_i", [nblk * k * P, HD], f32, kind="ExternalInput")
    s_i = nc.dram_tensor("s_i", [nblk, P, k * H], f32, kind="ExternalInput")
    d_i = nc.dram_tensor("d_i", [nblk, P, k], cmp_dt, kind="ExternalInput")
    iota_i = nc.dram_tensor("iota_i", [P, P], cmp_dt, kind="ExternalInput")
    out_o = nc.dram_tensor("out_o", [nblk * P, HD], f32, kind="ExternalOutput")

    # DRAM view of ft as [nblk, P(partition), k, HD]
    ft_v = ft_i[:].rearrange("(b t p) d -> b p t d", t=k, p=P)

    with tile.TileContext(nc) as tc:
        with (
            tc.tile_pool(name="const", bufs=1) as cp,
            tc.tile_pool(name="ftp", bufs=4) as ftp,
            tc.tile_pool(name="meta", bufs=4) as mp,
            tc.tile_pool(name="work", bufs=3) as wp,
            tc.tile_pool(name="ohp", bufs=3) as ohp,
            tc.tile_pool(name="valp", bufs=3) as vp,
            tc.tile_pool(name="outp", bufs=3) as op_,
            tc.tile_pool(name="psum", bufs=4, space="PSUM") as pp,
        ):
            iota_t = cp.tile([P, P], cmp_dt)
            nc.sync.dma_start(out=iota_t[:], in_=iota_i[:])

            for _rep in range(reps):
                for b in range(nblk):
                    ft_blk = ftp.tile([P, k, HD], f32)
                    nc.sync.dma_start(out=ft_blk[:], in_=ft_v[b])
                    s_blk = mp.tile([P, k * H], f32)
                    nc.sync.dma_start(out=s_blk[:], in_=s_i[b])
                    d_blk = mp.tile([P, k], cmp_dt)
                    nc.sync.dma_start(out=d_blk[:], in_=d_i[b])

                    # vals layout [P, k, 260]: cols 0:256 = ex*ft, 256:260 = ex
                    vals_blk = vp.tile([P, k, HD + H], f32)

                    # ex = exp(leaky_relu(s)); exp writes straight into the
                    # trailing 4 columns of each tile's vals slab
                    e_t = wp.tile([P, k * H], f32)
                    if act_lrelu:
                        nc.scalar.activation(
                            e_t[:], s_blk[:],
                            mybir.ActivationFunctionType.Lrelu, alpha=0.01,
                        )
                    else:
                        nc.vector.scalar_tensor_tensor(
                            out=e_t[:], in0=s_blk[:], scalar=0.01, in1=s_blk[:],
                            op0=mybir.AluOpType.mult, op1=mybir.AluOpType.max,
                        )
                    ex_t = wp.tile([P, k * H], f32, tag="ex_t")
                    nc.scalar.activation(
                        ex_t[:], e_t[:], mybir.ActivationFunctionType.Exp
                    )
                    ex_blk = vals_blk[:, :, HD : HD + H]
                    nc.vector.tensor_copy(
                        out=ex_blk, in_=ex_t[:].rearrange("p (t h) -> p t h", h=H)
                    )

                    # one-hot for all k tiles in one op:
                    # oh[e, t, n] = (iota[n] == dstl[e, t])
                    oh_blk = ohp.tile([P, k, P], f32)
                    nc.vector.tensor_tensor(
                        out=oh_blk[:],
                        in0=iota_t[:, None, :].to_broadcast([P, k, P]),
                        in1=d_blk[:, :, None].to_broadcast([P, k, P]),
                        op=mybir.AluOpType.is_equal,
                    )

                    # vals[e, t, h, d] = ft[e, t, h, d] * ex[e, t, h]
                    kd = k - gp_tiles
                    nc.vector.tensor_tensor(
                        out=vals_blk[:, :kd, :HD].rearrange(
                            "p t (h d) -> p t h d", h=H
                        ),
                        in0=ft_blk[:, :kd].rearrange("p t (h d) -> p t h d", h=H),
                        in1=ex_blk[:, :kd, :, None].to_broadcast([P, kd, H, D]),
                        op=mybir.AluOpType.mult,
                    )
                    if gp_tiles:
                        nc.gpsimd.tensor_tensor(
                            out=vals_blk[:, kd:, :HD].rearrange(
                                "p t (h d) -> p t h d", h=H
                            ),
                            in0=ft_blk[:, kd:].rearrange(
                                "p t (h d) -> p t h d", h=H
                            ),
                            in1=ex_blk[:, kd:, :, None].to_broadcast(
                                [P, gp_tiles, H, D]
                            ),
                            op=mybir.AluOpType.mult,
                        )

                    # single matmul per tile accumulates num (0:256) + den
                    # (256:260) into one PSUM bank
                    acc = pp.tile([P, HD + H], f32, tag="acc")
                    for t in range(k):
                        nc.tensor.matmul(
                            acc[:], lhsT=oh_blk[:, t, :], rhs=vals_blk[:, t],
                            start=(t == 0), stop=(t == k - 1),
                        )

                    den = wp.tile([P, H], f32, tag="den")
                    nc.vector.tensor_scalar_add(den[:], acc[:, HD : HD + H], 1e-30)
                    rec = wp.tile([P, H], f32, tag="rec")
                    nc.vector.reciprocal(rec[:], den[:])
                    outsb = op_.tile([P, H, D], f32)
                    nc.vector.tensor_tensor(
                        out=outsb[:],
                        in0=acc[:, :HD].rearrange("p (h d) -> p h d", h=H),
                        in1=rec[:, :, None].to_broadcast([P, H, D]),
                        op=mybir.AluOpType.mult,
                    )
                    nc.sync.dma_start(
                        out=out_o[b * P : (b + 1) * P, :],
                        in_=outsb[:].rearrange("p h d -> p (h d)"),
                    )

    nc.compile()
    return nc


def kernel(a1, a2, ft, dst):
    global LAST_RESULT, LAST_NC, LAST_IN_MAPS
    a1 = np.asarray(a1, dtype=np.float32)
    a2 = np.asarray(a2, dtype=np.float32)
    ft = np.asarray(ft, dtype=np.float32)
    dst = np.asarray(dst)

    n = a1.shape[0]
    e = dst.shape[0]
    assert a1.shape == (n, H, 1) and a2.shape == (e, H, 1)
    assert ft.shape == (e, H, D)

    # ---- host prep: sort edges by dst, fold a1 gather ----
    order = np.argsort(dst, kind="stable")
    dst_s = dst[order].astype(np.int64)
    s_all = (a1[:, :, 0][dst_s] + a2[order, :, 0]).astype(np.float32)  # [E,H]
    ft_s = ft[order].reshape(e, HD)  # [E, 256]

    nblk_total = math.ceil(n / P)                      # 391
    nblk = math.ceil(nblk_total / N_CORES)             # 49 blocks per core
    npc = nblk * P                                     # 6272 nodes per core

    # edges per 128-node block (global)
    block_starts = np.searchsorted(dst_s, np.arange(0, (nblk * N_CORES) * P + 1, P))
    counts = np.diff(block_starts)                     # [nblk*8]
    k = max(1, int(math.ceil(counts.max() / P)))       # edge tiles per block
    epb = k * P                                        # padded edges per block

    # ---- pack per-core inputs ----
    if OH_BF16:
        import ml_dtypes

        cmp_np = np.dtype(ml_dtypes.bfloat16)
    else:
        cmp_np = np.dtype(np.float32)
    iota_np = np.broadcast_to(
        np.arange(P, dtype=cmp_np)[None, :], (P, P)
    ).copy()

    in_maps = []
    for c in range(N_CORES):
        ftp = np.zeros((nblk * epb, HD), dtype=np.float32)
        sp = np.full((nblk * epb, H), -1e9, dtype=np.float32)
        dp = np.zeros((nblk * epb,), dtype=np.float32)
        for bl in range(nblk):
            g = c * nblk + bl                          # global block id
            lo, hi = block_starts[g], block_starts[g + 1]
            cnt = hi - lo
            o = bl * epb
            ftp[o : o + cnt] = ft_s[lo:hi]
            sp[o : o + cnt] = s_all[lo:hi]
            dp[o : o + cnt] = (dst_s[lo:hi] - g * P).astype(np.float32)
        # swizzle: [nblk, k, P, x] -> [nblk, P, k, x]
        s_sw = np.ascontiguousarray(
            sp.reshape(nblk, k, P, H).transpose(0, 2, 1, 3)
        ).reshape(nblk, P, k * H)
        d_sw = np.ascontiguousarray(
            dp.reshape(nblk, k, P).transpose(0, 2, 1)
        ).astype(cmp_np)
        in_maps.append({"ft_i": ftp, "s_i": s_sw, "d_i": d_sw, "iota_i": iota_np})

    key = (nblk, k, OH_BF16, GP_TILES)
    if key not in _kernel_cache:
        _kernel_cache[key] = _build(nblk, k, bf16_oh=OH_BF16, gp_tiles=GP_TILES)
    nc = _kernel_cache[key]

    try:
        res = run_bass_kernel_spmd(nc, in_maps, core_ids=list(range(N_CORES)))
    except Exception:
        # transient NRT_EXEC_UNIT_UNRECOVERABLE has been observed once on a
        # shared device; one retry clears it
        res = run_bass_kernel_spmd(nc, in_maps, core_ids=list(range(N_CORES)))
    LAST_RESULT = res
    LAST_NC = nc
    LAST_IN_MAPS = in_maps

    out = np.empty((n, H, D), dtype=np.float32)
    for c in range(N_CORES):
        lo = c * npc
        real = min(npc, n - lo)
        if real <= 0:
            break
        out[lo : lo + real] = res.results[c]["out_o"].reshape(npc, H, D)[:real]
    return out
